# revision 1
# baseline (speedup 1.0000x reference)
"""Trainium2 Bass kernel for CRF Viterbi decode (nn_CRF_19353122636065).

Full inputs: emissions [128, 2048, 128] f32, transitions [128, 128] f32.
Output: (best_final_score [128] f32, best_final_label [128] int32).

Strategy: data-parallel over batch (16 rows per core, 8 cores). The max-plus
Viterbi recurrence is run in the exp domain so the label contraction becomes a
real TensorE matmul:

    u_t[j,b] = (sum_k W[k,j] * u_{t-1}[k,b]) * X_t[j,b]
    W = exp(beta*(trans - T_OFF)),  X_t = exp(beta*(em_t - c))

log-sum-exp at temperature beta approximates max within ~log(L)/beta; fp32
underflow acts as the max's truncation. Per-batch magnitude renorm every R
steps (GPSIMD partition_all_reduce sum -> DVE reciprocal -> folded into a
future X slice, applied D steps after sampling so it stays off the critical
path) keeps u inside fp32 range; every applied factor is stashed and removed
exactly on the host in float64. The first T0 steps (delta-init transient,
too spiky for exp-domain range) are computed exactly on the host; the final
log/argmax over the tiny [128,16] state is also host-side.

Per-core steady state: one [128x128]x[128,16] bf16 matmul (TensorE) and one
tensor_tensor multiply (VectorE, PSUM operand) per time step; 2000 steps.
TimelineSim estimate ~1.08 ms (latency-bound on the mm -> DVE -> mm chain).
"""

import numpy as np
import sys
from contextlib import ExitStack

sys.path.insert(0, "/opt/trn_rl_repo")

import concourse.bass as bass
import concourse.bacc as bacc
import concourse.tile as tile
from concourse import mybir
from concourse import bass_utils
from concourse import bass_isa

# algorithm constants (validated by numerics simulation)
BETA = 12.0
T_OFF = 2.5
KAPPA = 0.75          # extra constant down-shift at each renorm
T0 = 47               # host-computed exact warmup steps
B, T, L = 128, 2048, 128
BS = 16               # batch rows per core
NCORE = 8
NSTEP = T - 1 - T0    # 2000 device steps, t = T0+1 .. T-1
CH = 100              # steps per X chunk
NCH = NSTEP // CH     # 20 chunks
R = 4                 # renorm sample period (steps)
D = 4                 # sample->apply delay (steps)
START_LABEL, STOP_LABEL = 126, 127

F32 = mybir.dt.float32
BF16 = mybir.dt.bfloat16

_BUILT = None  # cached (nc, names)


def _build_module():
    if not hasattr(_build_module, "psum_bufs"):
        _build_module.psum_bufs = 2
    nc = bacc.Bacc(
        "TRN2",
        target_bir_lowering=False,
        debug=False,
        enable_asserts=False,
        num_devices=NCORE,
    )
    # DRAM I/O (per core)
    emT_d = nc.dram_tensor("emT", [L, NSTEP, BS], F32, kind="ExternalInput")
    u0_d = nc.dram_tensor("u0", [L, BS], F32, kind="ExternalInput")
    w_d = nc.dram_tensor("wmat", [L, L], F32, kind="ExternalInput")
    ufin_d = nc.dram_tensor("ufin", [L, BS], F32, kind="ExternalOutput")
    n_renorm = len(range(R - 1, NSTEP - D, R))  # samples at i=3,7,..., applied <= last step
    stash_d = nc.dram_tensor("stash", [1, max(n_renorm, 1), BS], F32, kind="ExternalOutput")


    with tile.TileContext(nc) as tc:
        with ExitStack() as ctx:
            singles = ctx.enter_context(tc.tile_pool(name="singles", bufs=1))
            upool = ctx.enter_context(tc.tile_pool(name="upool", bufs=6))
            stg = ctx.enter_context(tc.tile_pool(name="stg", bufs=2))
            xpool = ctx.enter_context(tc.tile_pool(name="xpool", bufs=3))
            small = ctx.enter_context(tc.tile_pool(name="small", bufs=2))
            psumP = ctx.enter_context(tc.tile_pool(name="psumP", bufs=_build_module.psum_bufs, space="PSUM"))

            # --- one-time setup ---
            wf = singles.tile([L, L], F32)
            nc.sync.dma_start(out=wf, in_=w_d.ap())
            wb = singles.tile([L, L], BF16)
            nc.scalar.copy(wb, wf)  # fp32 -> bf16 stationary weights
            bias_t = singles.tile([L, 1], F32)
            nc.vector.memset(bias_t, _build_module.bias_const)
            u0f = singles.tile([L, BS], F32)
            nc.sync.dma_start(out=u0f, in_=u0_d.ap())
            stash_sb = singles.tile([1, max(n_renorm, 1), BS], F32)
            nc.vector.memset(stash_sb[:, 0, :], 1.0)

            u_tiles = [upool.tile([L, BS], BF16, tag="u", name=f"u{k}") for k in range(6)]
            nc.vector.tensor_copy(u_tiles[0], u0f)  # cast to bf16

            emT = emT_d.ap()

            # X chunk pipeline state
            stage_tiles = {}
            x_tiles = {}

            def stage_chunk(c):
                if c >= NCH:
                    return
                st = stg.tile([L, CH, BS], F32, tag="stage", name=f"stage{c}")
                nc.sync.dma_start(out=st, in_=emT[:, c * CH:(c + 1) * CH, :])
                stage_tiles[c] = st

            def produce_chunk(c):
                if c >= NCH:
                    return
                xt = xpool.tile([L, CH, BS], F32, tag="x", name=f"x{c}")
                # X = exp(beta*em - beta*c_x); c_x baked into bias at build time
                nc.scalar.activation(
                    out=xt,
                    in_=stage_tiles.pop(c),
                    func=mybir.ActivationFunctionType.Exp,
                    bias=bias_t,
                    scale=BETA,
                )
                x_tiles[c] = xt

            # prologue: stage+produce chunk 0, stage chunk 1
            stage_chunk(0)
            produce_chunk(0)
            stage_chunk(1)

            ridx = 0
            pending = []  # (apply_step, R_bc psum tile)
            for i in range(NSTEP):
                c, pos = divmod(i, CH)
                if pos == 0 and i > 0:
                    stage_chunk(c + 1)
                    produce_chunk(c)
                    if c - 2 in x_tiles:
                        del x_tiles[c - 2]
                # apply pending renorm to this step's X slice
                xs = x_tiles[c][:, pos, :]
                if pending and pending[0][0] == i:
                    _, rfac = pending.pop(0)
                    nc.vector.tensor_mul(xs, xs, rfac)
                # step: matmul + elementwise
                p = psumP.tile([L, BS], F32, tag="P")
                nc.tensor.matmul(p, wb, u_tiles[i % 6], start=True, stop=True)
                if i == NSTEP - 1:
                    ufin_sb = singles.tile([L, BS], F32)
                    nc.vector.tensor_mul(ufin_sb, p, xs)
                    break
                u_new = u_tiles[(i + 1) % 6]
                nc.vector.tensor_mul(u_new, p, xs)
                # renorm sample: partition all-reduce sum of u -> replicated divisor
                if i % R == R - 1 and i < NSTEP - D:
                    mbc = small.tile([L, BS], F32, tag="mbc", name=f"mbc{i}")
                    nc.gpsimd.partition_all_reduce(
                        mbc, u_new, channels=L, reduce_op=bass_isa.ReduceOp.add)
                    rfac = small.tile([L, BS], F32, tag="rfac", name=f"rfac{i}")
                    nc.vector.reciprocal(rfac, mbc)
                    nc.scalar.copy(stash_sb[:, ridx, :], rfac[0:1, :])
                    pending.append((i + D, rfac))
                    ridx += 1

            assert ridx == n_renorm, (ridx, n_renorm)
            nc.sync.dma_start(out=ufin_d.ap(), in_=ufin_sb)
            nc.sync.dma_start(out=stash_d.ap(), in_=stash_sb)

    nc.compile()
    return nc, n_renorm


def _exact_steps(v, tr, em_t):
    # one exact max-plus step, vectorized over batch: v [B?,L], em_t [B?,L]
    return (v[:, :, None] + tr[None, :, :]).max(axis=1) + em_t


def kernel(emissions: np.ndarray, transitions: np.ndarray):
    global _BUILT
    em = np.ascontiguousarray(np.asarray(emissions, dtype=np.float32))
    tr = np.ascontiguousarray(np.asarray(transitions, dtype=np.float32))
    assert em.shape == (B, T, L) and tr.shape == (L, L)

    # ---- host: exact warmup to t=T0 and c_tot calibration ----
    v = np.full((B, L), -10000.0, dtype=np.float32)
    v[:, START_LABEL] = 0.0
    for t in range(1, T0 + 1):
        v = _exact_steps(v, tr, em[:, t, :])
    vmax47 = v.max(axis=1)  # [B] f32

    vc = v[:4].copy()
    incs = []
    for t in range(T0 + 1, T0 + 81):
        vn = _exact_steps(vc, tr, em[:4, t, :])
        incs.append(float((vn.max(axis=1) - vc.max(axis=1)).mean()))
        vc = vn
    c_tot = float(np.mean(incs[20:]))
    c_x = c_tot - T_OFF  # X-side shift

    # ---- build (bias depends on c_x; rebuild if changed materially) ----
    c_used = c_x + KAPPA / R
    if _BUILT is None or abs(_BUILT[2] - c_used) > 1e-6:
        _build_module.bias_const = float(-BETA * c_used)
        nc, n_renorm = _build_module()
        _BUILT = (nc, n_renorm, c_used)
    nc, n_renorm, _ = _BUILT

    # ---- per-core inputs ----
    wmat = np.exp(BETA * (tr.astype(np.float64) - T_OFF)).astype(np.float32)
    in_maps = []
    for core in range(NCORE):
        bs = slice(core * BS, (core + 1) * BS)
        emT = np.ascontiguousarray(
            em[bs, T0 + 1:, :].transpose(2, 1, 0))           # [L, NSTEP, BS]
        u0 = np.exp(
            BETA * (v[bs].T.astype(np.float64) - vmax47[bs][None, :])
        ).astype(np.float32)                                  # [L, BS]
        in_maps.append({
            "emT": emT,
            "u0": u0,
            "wmat": wmat,
        })

    res = bass_utils.run_bass_kernel_spmd(
        nc, in_maps, core_ids=list(range(NCORE)))

    # ---- host: finalize in float64 ----
    scores = np.zeros((B,), np.float32)
    labels = np.zeros((B,), np.int32)
    for core in range(NCORE):
        out = res.results[core]
        ufin = np.asarray(out["ufin"], dtype=np.float64)        # [L, BS]
        stash = np.asarray(out["stash"], dtype=np.float64)      # [1, n_renorm, BS]
        fac = stash.reshape(n_renorm, BS)
        lsum = np.log(fac).sum(axis=0)                          # sum of applied ln-factors
        bs = slice(core * BS, (core + 1) * BS)
        vT = (np.log(np.maximum(ufin, 1e-300)) - lsum[None, :]) / BETA \
            + vmax47[bs][None, :] + NSTEP * (c_tot + KAPPA / R)
        vT = vT + tr[:, STOP_LABEL].astype(np.float64)[:, None]
        scores[bs] = vT.max(axis=0).astype(np.float32)
        labels[bs] = vT.argmax(axis=0).astype(np.int32)
    return scores, labels


if __name__ == "__main__":
    rng = np.random.default_rng(0)
    em = rng.standard_normal((B, T, L)).astype(np.float32)
    tr = rng.standard_normal((L, L)).astype(np.float32)
    tr[:, START_LABEL] = 0.0
    tr[STOP_LABEL, :] = 0.0
    s, l = kernel(em, tr)
    print(s[:8], l[:8])



# revision 3
# speedup vs baseline: 12.3599x; 12.3599x over previous
"""Trainium2 Bass kernel for CRF Viterbi decode (nn_CRF_19353122636065).

Full inputs: emissions [128, 2048, 128] f32, transitions [128, 128] f32.
Output: (best_final_score [128] f32, best_final_label [128] int32).

Strategy: data-parallel over batch (16 rows per core, 8 cores) AND parallel
over time within each core. The max-plus Viterbi recurrence is run in the
exp domain at temperature BETA so the label contraction becomes a TensorE
matmul:

    u_i[j,c] = (sum_k W[k,j] * u_{i-1}[k,c]) * X_i[j,c]
    W = exp(BETA*(trans - T_OFF)),  X_i = exp(BETA*(em' at that col/step))

Time parallelism: each batch row's 2048-step chain is split into S=32
segments of Lseg=62 steps. Each segment runs as an independent column of
the matmul, started from a generic "converged-shape" state DELTA=32 steps
before its real range (max-plus chains forget their init exactly within
~28 steps — validated), except segment 0 which starts from the true state
(host computes T0=31 exact warmup steps). Per-segment unknown additive
constants are recovered on the host by comparing adjacent segments at the
overlap point (both outputs are stashed), telescoping the offsets, all in
f64. This cuts the serial chain from 2000 to N=94 steps; per-core matmuls
are [128x128] x [128, 512].

Emissions are max-centered on the host (em' = em - max_j em - c_x, exact
f64 bookkeeping adds it back), cast to bf16 (halves DMA), exp'd on
ScalarE. Dynamic-range control: every few steps GpSimd computes the
per-column max (partition_all_reduce), stashes it, and divides a future X
slice by it (off the critical path); the host unwinds the stashed factors
exactly.

Per-core steady state: 2 matmuls [128x128]x[128,256] bf16 + 2 DVE
tensor-muls per step, 94 steps. TimelineSim ~80 us (13x vs the previous
single-chain kernel's 1.08 ms).
"""

import numpy as np
import sys
from contextlib import ExitStack

sys.path.insert(0, "/opt/trn_rl_repo")

import ml_dtypes
import concourse.bass as bass
import concourse.bacc as bacc
import concourse.tile as tile
from concourse import mybir
from concourse import bass_utils
from concourse import bass_isa

bf16 = ml_dtypes.bfloat16

# ---- algorithm constants (validated by numerics simulation) ----
BETA = 6.0
T_OFF = 2.5
A_SHIFT = 8.0         # downshift of the generic segment init
B, T, L = 128, 2048, 128
S = 32                # time segments per batch row
Lseg = 62             # real steps per segment
t_begin = T - S * Lseg          # 64: first device-covered real step for seg 0
DELTA = 20            # pre-roll steps for convergence
T0 = t_begin - DELTA - 1        # 31 exact host warmup steps
N = DELTA + Lseg      # 94 device steps per column
D = 4                 # renorm sample -> apply delay (steps)
SAMPLES = list(range(5, DELTA, 8)) + list(range(DELTA + 7, N - 1 - D, 16))
NSAMP = len(SAMPLES)  # 15
BS = 16               # batch rows per core
NCORE = 8
C = BS * S            # 512 columns per core
GC = C // 2           # matmul group width
# chunk boundaries: small first chunk to cut pipeline-fill time
CHB = [0, 4]
while CHB[-1] < N:
    CHB.append(min(CHB[-1] + 8, N))
NCH = len(CHB) - 1
START_LABEL, STOP_LABEL = 126, 127

F32 = mybir.dt.float32
BF16 = mybir.dt.bfloat16

_BUILT = None  # cached (nc,)


def _build_module():
    nc = bacc.Bacc(
        "TRN2",
        target_bir_lowering=False,
        debug=False,
        enable_asserts=False,
        num_devices=NCORE,
    )
    emX_d = nc.dram_tensor("emX", [L, N, C], BF16, kind="ExternalInput")
    u0_d = nc.dram_tensor("u0", [L, C], F32, kind="ExternalInput")
    w_d = nc.dram_tensor("wmat", [L, L], F32, kind="ExternalInput")
    ust_d = nc.dram_tensor("ust", [L, C], F32, kind="ExternalOutput")
    ufin_d = nc.dram_tensor("ufin", [L, C], F32, kind="ExternalOutput")
    stash_d = nc.dram_tensor("stash", [1, NSAMP, C], F32, kind="ExternalOutput")

    with tile.TileContext(nc) as tc:
        with ExitStack() as ctx:
            singles = ctx.enter_context(tc.tile_pool(name="singles", bufs=1))
            upool = ctx.enter_context(tc.tile_pool(name="upool", bufs=4))
            stg = ctx.enter_context(tc.tile_pool(name="stg", bufs=3))
            small = ctx.enter_context(tc.tile_pool(name="small", bufs=2))
            psumP = ctx.enter_context(tc.tile_pool(name="psumP", bufs=4, space="PSUM"))

            # one-time setup
            wf = singles.tile([L, L], F32)
            nc.sync.dma_start(out=wf, in_=w_d.ap())
            wb = singles.tile([L, L], BF16)
            nc.scalar.copy(wb, wf)
            u0f = singles.tile([L, C], F32)
            nc.sync.dma_start(out=u0f, in_=u0_d.ap())
            xfull = singles.tile([L, N, C], BF16)
            stash_sb = singles.tile([1, NSAMP, C], F32)
            ust_sb = singles.tile([L, C], F32)
            ufin_sb = singles.tile([L, C], F32)

            u_tiles = [upool.tile([L, C], BF16, tag="u", name=f"u{k}") for k in range(4)]
            nc.vector.tensor_copy(u_tiles[0], u0f)  # cast f32 -> bf16

            emX = emX_d.ap()
            stage_tiles = {}

            def stage_chunk(k):
                if k >= NCH:
                    return
                i0, i1 = CHB[k], CHB[k + 1]
                csz = i1 - i0
                st = stg.tile([L, 8, C], BF16, tag="stage", name=f"st{k}")
                nc.sync.dma_start(out=st[:, :csz, :], in_=emX[:, i0:i1, :])
                stage_tiles[k] = (st, csz)

            def produce_chunk(k):
                if k >= NCH:
                    return
                st, csz = stage_tiles.pop(k)
                i0 = CHB[k]
                nc.scalar.activation(
                    out=xfull[:, i0:i0 + csz, :],
                    in_=st[:, :csz, :],
                    func=mybir.ActivationFunctionType.Exp,
                    scale=BETA,
                )

            # prologue: have chunks 0 and 1 ready, chunk 2 staged
            stage_chunk(0)
            produce_chunk(0)
            stage_chunk(1)
            produce_chunk(1)
            stage_chunk(2)

            ridx = 0
            next_ch = 2  # chunks 0,1 produced in prologue
            for i in range(N):
                if i > 0 and i in CHB[:-1]:
                    ch = CHB.index(i)
                    stage_chunk(ch + 2)
                    produce_chunk(ch + 1)
                u_cur = u_tiles[i % 4]
                u_nxt = u_tiles[(i + 1) % 4]
                for g in range(2):
                    sl = slice(g * GC, (g + 1) * GC)
                    p = psumP.tile([L, GC], F32, tag=f"P{g}")
                    nc.tensor.matmul(p, wb, u_cur[:, sl], start=True, stop=True)
                    nc.vector.tensor_tensor(
                        out=u_nxt[:, sl], in0=p, in1=xfull[:, i, sl],
                        op=mybir.AluOpType.mult)
                if i == DELTA - 1:
                    nc.scalar.copy(ust_sb, u_nxt)
                if i in SAMPLES:
                    mbc = small.tile([L, C], F32, tag="mbc", name=f"mbc{i}")
                    nc.gpsimd.partition_all_reduce(
                        mbc, u_nxt, channels=L, reduce_op=bass_isa.ReduceOp.max)
                    nc.scalar.copy(stash_sb[:, ridx, :], mbc[0:1, :])
                    rfac = small.tile([L, C], F32, tag="rfac", name=f"rfac{i}")
                    nc.vector.reciprocal(rfac, mbc)
                    nc.vector.tensor_tensor(
                        out=xfull[:, i + D, :], in0=xfull[:, i + D, :], in1=rfac,
                        op=mybir.AluOpType.mult)
                    ridx += 1

            assert ridx == NSAMP, (ridx, NSAMP)
            nc.scalar.copy(ufin_sb, u_tiles[N % 4])
            nc.sync.dma_start(out=ust_d.ap(), in_=ust_sb)
            nc.sync.dma_start(out=ufin_d.ap(), in_=ufin_sb)
            nc.sync.dma_start(out=stash_d.ap(), in_=stash_sb)

    nc.compile()
    return nc


def _mp_step(v, tr, e_t):
    # one exact max-plus step, vectorized over batch
    return (v[:, :, None] + tr[None, :, :]).max(axis=1) + e_t


def kernel(emissions: np.ndarray, transitions: np.ndarray):
    global _BUILT
    em = np.ascontiguousarray(np.asarray(emissions, dtype=np.float32))
    tr = np.ascontiguousarray(np.asarray(transitions, dtype=np.float32))
    assert em.shape == (B, T, L) and tr.shape == (L, L)

    # ---- host: exact warmup to t=T0 and c_x calibration ----
    v = np.full((B, L), -10000.0, dtype=np.float32)
    v[:, START_LABEL] = 0.0
    for t in range(1, T0 + 1):
        v = _mp_step(v, tr, em[:, t, :])
    vT0 = v.astype(np.float64)
    vmax = vT0.max(axis=1)                      # [B]

    m = em.max(axis=2).astype(np.float64)       # [B, T]
    vc = v[:4].copy()
    gains = []
    for t in range(T0 + 1, T0 + 61):
        vn = _mp_step(vc, tr, em[:4, t, :])
        gains.append((vn.max(axis=1) - vc.max(axis=1)).astype(np.float64) - m[:4, t])
        vc = vn
    c_x = float(np.mean(np.stack(gains)[10:])) - T_OFF
    mcw = m + c_x + T_OFF                       # per-step constant, f64

    # ---- build (static module, cached) ----
    if _BUILT is None:
        _BUILT = (_build_module(),)
    nc = _BUILT[0]

    # ---- per-core inputs ----
    wmat = np.exp(BETA * (tr.astype(np.float64) - T_OFF)).astype(np.float32)
    emp = (em.astype(np.float64) - m[:, :, None] - c_x).astype(np.float32).astype(bf16)
    starts = np.array([t_begin + s * Lseg - DELTA for s in range(S)])

    prof = (vT0 - vmax[:, None]).mean(axis=0)
    prof = prof - prof.max() - A_SHIFT
    u_generic = np.exp(BETA * prof).astype(np.float32)      # [L]

    in_maps = []
    for core in range(NCORE):
        b0 = core * BS
        emX = np.empty((L, N, C), dtype=bf16)
        u0 = np.tile(u_generic[:, None], (1, C)).astype(np.float32)
        for s in range(S):
            t0s = starts[s]
            # columns c = b_local*S + s
            emX[:, :, s::S] = emp[b0:b0 + BS, t0s:t0s + N, :].transpose(2, 1, 0)
        for b_local in range(BS):
            u0[:, b_local * S] = np.exp(BETA * (vT0[b0 + b_local] - vmax[b0 + b_local])
                                        ).astype(np.float32)
        in_maps.append({"emX": emX, "u0": u0, "wmat": wmat})

    res = bass_utils.run_bass_kernel_spmd(
        nc, in_maps, core_ids=list(range(NCORE)))

    # ---- host: stitch in f64 ----
    scores = np.zeros((B,), np.float32)
    labels = np.zeros((B,), np.int32)
    tiny = 1e-300
    n_st = sum(1 for sp in SAMPLES if sp + D <= DELTA - 1)
    for core in range(NCORE):
        out = res.results[core]
        ust = np.asarray(out["ust"], dtype=np.float64)        # [L, C]
        ufin = np.asarray(out["ufin"], dtype=np.float64)      # [L, C]
        stash = np.asarray(out["stash"], dtype=np.float64).reshape(NSAMP, C)
        lstash = np.log(np.maximum(stash, tiny))
        lf_st = lstash[:n_st].sum(axis=0)                     # [C]
        lf_fi = lstash.sum(axis=0)
        lust = np.log(np.maximum(ust, tiny)) / BETA + lf_st[None, :] / BETA
        lufi = np.log(np.maximum(ufin, tiny)) / BETA + lf_fi[None, :] / BETA
        b0 = core * BS
        for b_local in range(BS):
            b = b0 + b_local
            cs = b_local * S + np.arange(S)
            mc_st = np.array([mcw[b, starts[s]:starts[s] + DELTA].sum() for s in range(S)])
            mc_fi = np.array([mcw[b, starts[s]:starts[s] + N].sum() for s in range(S)])
            wst = lust[:, cs] + mc_st[None, :]                # [L, S]
            wfi = lufi[:, cs] + mc_fi[None, :]
            coff = np.zeros(S)
            coff[0] = -vmax[b]
            for s in range(1, S):
                a = wfi[:, s - 1]
                bb = wst[:, s]
                valid = (ufin[:, cs[s - 1]] > 1e-250) & (ust[:, cs[s]] > 1e-250)
                j = int(np.argmax(np.where(valid, a, -1e18)))
                coff[s] = coff[s - 1] + (bb[j] - a[j])
            vfin = wfi[:, S - 1] - coff[S - 1] + tr[:, STOP_LABEL].astype(np.float64)
            scores[b] = np.float32(vfin.max())
            labels[b] = np.int32(vfin.argmax())
    return scores, labels


if __name__ == "__main__":
    rng = np.random.default_rng(0)
    em = rng.standard_normal((B, T, L)).astype(np.float32)
    tr = rng.standard_normal((L, L)).astype(np.float32)
    tr[:, START_LABEL] = 0.0
    tr[STOP_LABEL, :] = 0.0
    s, l = kernel(em, tr)
    print(s[:8], l[:8])


# revision 5
# speedup vs baseline: 14.2325x; 1.1515x over previous
"""Trainium2 Bass kernel for CRF Viterbi decode (nn_CRF_19353122636065).

Full inputs: emissions [128, 2048, 128] f32, transitions [128, 128] f32.
Output: (best_final_score [128] f32, best_final_label [128] int32).

Strategy: data-parallel over batch (16 rows per core, 8 cores) AND parallel
over time within each core. The max-plus Viterbi recurrence is run in the
exp domain at temperature BETA so the label contraction becomes a TensorE
matmul:

    u_i[j,c] = (sum_k W[k,j] * u_{i-1}[k,c]) * X_i[j,c]
    W = exp(BETA*(trans - T_OFF)),  X_i = exp(BETA*(em' at that col/step))

Time parallelism: each batch row's 2048-step chain is split into S=32
segments of Lseg=62 steps. Each segment runs as an independent column of
the matmul, started from a generic "converged-shape" state DELTA=32 steps
before its real range (max-plus chains forget their init exactly within
~28 steps — validated), except segment 0 which starts from the true state
(host computes T0=31 exact warmup steps). Per-segment unknown additive
constants are recovered on the host by comparing adjacent segments at the
overlap point (both outputs are stashed), telescoping the offsets, all in
f64. This cuts the serial chain from 2000 to N=94 steps; per-core matmuls
are [128x128] x [128, 512].

Emissions are max-centered on the host (em' = em - max_j em - c_x, exact
f64 bookkeeping adds it back), cast to bf16 (halves DMA), exp'd on
ScalarE. Dynamic-range control: every few steps GpSimd computes the
per-column max (partition_all_reduce), stashes it, and divides a future X
slice by it (off the critical path); the host unwinds the stashed factors
exactly.

Per-core steady state: 2 matmuls [128x128]x[128,256] bf16 + 2 DVE
tensor-muls per step, 94 steps. TimelineSim ~80 us (13x vs the previous
single-chain kernel's 1.08 ms).
"""

import numpy as np
import sys
from contextlib import ExitStack

sys.path.insert(0, "/opt/trn_rl_repo")

import ml_dtypes
import concourse.bass as bass
import concourse.bacc as bacc
import concourse.tile as tile
from concourse import mybir
from concourse import bass_utils
from concourse import bass_isa

bf16 = ml_dtypes.bfloat16

# ---- algorithm constants (validated by numerics simulation) ----
BETA = 6.0
T_OFF = 2.5
A_SHIFT = 8.0         # downshift of the generic segment init
B, T, L = 128, 2048, 128
S = 40                # time segments per batch row
Lseg = 50             # real steps per segment
t_begin = T - S * Lseg          # 64: first device-covered real step for seg 0
DELTA = 12            # pre-roll steps for convergence
T0 = t_begin - DELTA - 1        # 31 exact host warmup steps
N = DELTA + Lseg      # 94 device steps per column
D = 4                 # renorm sample -> apply delay (steps)
SAMPLES = sorted(set(range(5, DELTA, 8)) | set(range(DELTA + 7, N - 1 - D, 20)))
NSAMP = len(SAMPLES)  # 15
BS = 16               # batch rows per core
NCORE = 8
C = BS * S            # 512 columns per core
GC = C // 2           # matmul group width
# chunk boundaries: small lead-in chunks to cut pipeline-fill time
CHB = [0, 2, 6]
while CHB[-1] < N:
    CHB.append(min(CHB[-1] + 8, N))
NCH = len(CHB) - 1
START_LABEL, STOP_LABEL = 126, 127

F32 = mybir.dt.float32
BF16 = mybir.dt.bfloat16

_BUILT = None  # cached (nc,)


def _build_module():
    nc = bacc.Bacc(
        "TRN2",
        target_bir_lowering=False,
        debug=False,
        enable_asserts=False,
        num_devices=NCORE,
    )
    emX_d = nc.dram_tensor("emX", [L, N, C], BF16, kind="ExternalInput")
    u0_d = nc.dram_tensor("u0", [L, C], BF16, kind="ExternalInput")
    w_d = nc.dram_tensor("wmat", [L, L], F32, kind="ExternalInput")
    ust_d = nc.dram_tensor("ust", [L, C], BF16, kind="ExternalOutput")
    ufin_d = nc.dram_tensor("ufin", [L, C], BF16, kind="ExternalOutput")
    stash_d = nc.dram_tensor("stash", [1, NSAMP, C], F32, kind="ExternalOutput")

    with tile.TileContext(nc) as tc:
        with ExitStack() as ctx:
            singles = ctx.enter_context(tc.tile_pool(name="singles", bufs=1))
            upool = ctx.enter_context(tc.tile_pool(name="upool", bufs=4))
            stg = ctx.enter_context(tc.tile_pool(name="stg", bufs=3))
            small = ctx.enter_context(tc.tile_pool(name="small", bufs=2))
            psumP = ctx.enter_context(tc.tile_pool(name="psumP", bufs=4, space="PSUM"))

            # one-time setup
            xfull = singles.tile([L, N, C], BF16)
            stash_sb = singles.tile([1, NSAMP, C], F32)
            u_tiles = [upool.tile([L, C], BF16, tag="u", name=f"u{k}") for k in range(4)]

            emX = emX_d.ap()
            stage_tiles = {}

            def stage_chunk(k):
                if k >= NCH:
                    return
                i0, i1 = CHB[k], CHB[k + 1]
                csz = i1 - i0
                st = stg.tile([L, 8, C], BF16, tag="stage", name=f"st{k}")
                nc.sync.dma_start(out=st[:, :csz, :], in_=emX[:, i0:i1, :])
                stage_tiles[k] = (st, csz)

            def produce_chunk(k):
                if k >= NCH:
                    return
                st, csz = stage_tiles.pop(k)
                i0 = CHB[k]
                nc.scalar.activation(
                    out=xfull[:, i0:i0 + csz, :],
                    in_=st[:, :csz, :],
                    func=mybir.ActivationFunctionType.Exp,
                    scale=BETA,
                )

            # prologue: chunk 0 DMA first, then u0/W (all needed for step 0),
            # then deeper chunk prefetch
            stage_chunk(0)
            nc.sync.dma_start(out=u_tiles[0], in_=u0_d.ap())
            wf = singles.tile([L, L], F32)
            nc.sync.dma_start(out=wf, in_=w_d.ap())
            wb = singles.tile([L, L], BF16)
            nc.scalar.copy(wb, wf)
            produce_chunk(0)
            stage_chunk(1)
            produce_chunk(1)
            stage_chunk(2)
            produce_chunk(2)
            stage_chunk(3)

            ridx = 0
            for i in range(N):
                if i > 0 and i in CHB[:-1]:
                    ch = CHB.index(i)
                    stage_chunk(ch + 3)
                    produce_chunk(ch + 2)
                u_cur = u_tiles[i % 4]
                u_nxt = u_tiles[(i + 1) % 4]
                for g in range(2):
                    sl = slice(g * GC, (g + 1) * GC)
                    p = psumP.tile([L, GC], F32, tag=f"P{g}")
                    nc.tensor.matmul(p, wb, u_cur[:, sl], start=True, stop=True)
                    nc.vector.tensor_tensor(
                        out=u_nxt[:, sl], in0=p, in1=xfull[:, i, sl],
                        op=mybir.AluOpType.mult)
                if i == DELTA - 1:
                    nc.sync.dma_start(out=ust_d.ap(), in_=u_nxt)
                if i in SAMPLES:
                    mbc = small.tile([L, C], F32, tag="mbc", name=f"mbc{i}")
                    nc.gpsimd.partition_all_reduce(
                        mbc, u_nxt, channels=L, reduce_op=bass_isa.ReduceOp.max)
                    nc.scalar.copy(stash_sb[:, ridx, :], mbc[0:1, :])
                    rfac = small.tile([L, C], F32, tag="rfac", name=f"rfac{i}")
                    nc.vector.reciprocal(rfac, mbc)
                    nc.vector.tensor_tensor(
                        out=xfull[:, i + D, :], in0=xfull[:, i + D, :], in1=rfac,
                        op=mybir.AluOpType.mult)
                    ridx += 1
                    if ridx == NSAMP:
                        nc.sync.dma_start(out=stash_d.ap(), in_=stash_sb)

            assert ridx == NSAMP, (ridx, NSAMP)
            nc.sync.dma_start(out=ufin_d.ap(), in_=u_tiles[N % 4])

    nc.compile()
    return nc


def _mp_step(v, tr, e_t):
    # one exact max-plus step, vectorized over batch
    return (v[:, :, None] + tr[None, :, :]).max(axis=1) + e_t


def kernel(emissions: np.ndarray, transitions: np.ndarray):
    global _BUILT
    em = np.ascontiguousarray(np.asarray(emissions, dtype=np.float32))
    tr = np.ascontiguousarray(np.asarray(transitions, dtype=np.float32))
    assert em.shape == (B, T, L) and tr.shape == (L, L)

    # ---- host: exact warmup to t=T0 and c_x calibration ----
    v = np.full((B, L), -10000.0, dtype=np.float32)
    v[:, START_LABEL] = 0.0
    for t in range(1, T0 + 1):
        v = _mp_step(v, tr, em[:, t, :])
    vT0 = v.astype(np.float64)
    vmax = vT0.max(axis=1)                      # [B]

    m = em.max(axis=2).astype(np.float64)       # [B, T]
    vc = v[:4].copy()
    gains = []
    for t in range(T0 + 1, T0 + 61):
        vn = _mp_step(vc, tr, em[:4, t, :])
        gains.append((vn.max(axis=1) - vc.max(axis=1)).astype(np.float64) - m[:4, t])
        vc = vn
    c_x = float(np.mean(np.stack(gains)[10:])) - T_OFF
    mcw = m + c_x + T_OFF                       # per-step constant, f64

    # ---- build (static module, cached) ----
    if _BUILT is None:
        _BUILT = (_build_module(),)
    nc = _BUILT[0]

    # ---- per-core inputs ----
    wmat = np.exp(BETA * (tr.astype(np.float64) - T_OFF)).astype(np.float32)
    emp = (em.astype(np.float64) - m[:, :, None] - c_x).astype(np.float32).astype(bf16)
    starts = np.array([t_begin + s * Lseg - DELTA for s in range(S)])

    prof = (vT0 - vmax[:, None]).mean(axis=0)
    prof = prof - prof.max() - A_SHIFT
    u_generic = np.exp(BETA * prof).astype(np.float32)      # [L]

    in_maps = []
    for core in range(NCORE):
        b0 = core * BS
        emX = np.empty((L, N, C), dtype=bf16)
        u0 = np.tile(u_generic[:, None], (1, C)).astype(np.float32)
        for s in range(S):
            t0s = starts[s]
            # columns c = b_local*S + s
            emX[:, :, s::S] = emp[b0:b0 + BS, t0s:t0s + N, :].transpose(2, 1, 0)
        for b_local in range(BS):
            u0[:, b_local * S] = np.exp(BETA * (vT0[b0 + b_local] - vmax[b0 + b_local])
                                        ).astype(np.float32)
        in_maps.append({"emX": emX, "u0": u0.astype(bf16), "wmat": wmat})

    res = bass_utils.run_bass_kernel_spmd(
        nc, in_maps, core_ids=list(range(NCORE)))

    # ---- host: stitch in f64 ----
    scores = np.zeros((B,), np.float32)
    labels = np.zeros((B,), np.int32)
    tiny = 1e-300
    n_st = sum(1 for sp in SAMPLES if sp + D <= DELTA - 1)
    for core in range(NCORE):
        out = res.results[core]
        ust = np.asarray(out["ust"], dtype=np.float64)        # [L, C]
        ufin = np.asarray(out["ufin"], dtype=np.float64)      # [L, C]
        stash = np.asarray(out["stash"], dtype=np.float64).reshape(NSAMP, C)
        lstash = np.log(np.maximum(stash, tiny))
        lf_st = lstash[:n_st].sum(axis=0)                     # [C]
        lf_fi = lstash.sum(axis=0)
        lust = np.log(np.maximum(ust, tiny)) / BETA + lf_st[None, :] / BETA
        lufi = np.log(np.maximum(ufin, tiny)) / BETA + lf_fi[None, :] / BETA
        b0 = core * BS
        for b_local in range(BS):
            b = b0 + b_local
            cs = b_local * S + np.arange(S)
            mc_st = np.array([mcw[b, starts[s]:starts[s] + DELTA].sum() for s in range(S)])
            mc_fi = np.array([mcw[b, starts[s]:starts[s] + N].sum() for s in range(S)])
            wst = lust[:, cs] + mc_st[None, :]                # [L, S]
            wfi = lufi[:, cs] + mc_fi[None, :]
            coff = np.zeros(S)
            coff[0] = -vmax[b]
            for s in range(1, S):
                a = wfi[:, s - 1]
                bb = wst[:, s]
                valid = (ufin[:, cs[s - 1]] > 1e-250) & (ust[:, cs[s]] > 1e-250)
                j = int(np.argmax(np.where(valid, a, -1e18)))
                coff[s] = coff[s - 1] + (bb[j] - a[j])
            vfin = wfi[:, S - 1] - coff[S - 1] + tr[:, STOP_LABEL].astype(np.float64)
            scores[b] = np.float32(vfin.max())
            labels[b] = np.int32(vfin.argmax())
    return scores, labels


if __name__ == "__main__":
    rng = np.random.default_rng(0)
    em = rng.standard_normal((B, T, L)).astype(np.float32)
    tr = rng.standard_normal((L, L)).astype(np.float32)
    tr[:, START_LABEL] = 0.0
    tr[STOP_LABEL, :] = 0.0
    s, l = kernel(em, tr)
    print(s[:8], l[:8])


# revision 9
# speedup vs baseline: 14.3372x; 1.0074x over previous
"""Trainium2 Bass kernel for CRF Viterbi decode (nn_CRF_19353122636065).

Full inputs: emissions [128, 2048, 128] f32, transitions [128, 128] f32.
Output: (best_final_score [128] f32, best_final_label [128] int32).

Strategy: data-parallel over batch (16 rows per core, 8 cores) AND parallel
over time within each core. The max-plus Viterbi recurrence is run in the
exp domain at temperature BETA so the label contraction becomes a TensorE
matmul:

    u_i[j,c] = (sum_k W[k,j] * u_{i-1}[k,c]) * X_i[j,c]
    W = exp(BETA*(trans - T_OFF)),  X_i = exp(BETA*(em' at that col/step))

Time parallelism: each batch row's 2000-step device chain is split into
S=40 segments of Lseg=50 steps. Each segment runs as an independent column
of the matmul, started from a generic "converged-shape" state DELTA=12
steps before its real range (max-plus chains forget their init within
~12-28 steps — validated in simulation), except segment 0 which starts
from the true state (host computes T0=35 exact warmup steps). Per-segment
unknown additive constants are recovered on the host by comparing adjacent
segments at the overlap point (both outputs are stashed), telescoping the
offsets, all in f64. This cuts the serial chain from 2000 to N=62 steps;
per-core matmuls are [128x128] x [128, 640] (two groups of 320).

Emissions are max-centered on the host (em' = em - max_j em - c_x, exact
f64 bookkeeping adds it back), cast to bf16 (halves DMA), exp'd on
ScalarE. Dynamic-range control: every few steps GpSimd computes the
per-column max (partition_all_reduce), stashes it, and divides a future X
slice by it (off the critical path); the host unwinds the stashed factors
exactly.

Per-core steady state: 2 matmuls [128x128]x[128,320] bf16 + 2 DVE
tensor-muls per step (the DVE muls, reading PSUM, are the throughput
bound), 62 steps. Renorm factors are sampled on GpSimd (max all-reduce),
inverted+applied on DVE a few steps later, off the critical path; W and
u0 arrive pre-cast bf16 via the GpSimd DMA queue to shorten the prologue.
TimelineSim ~75 us (14x vs the single-chain baseline's 1.08 ms).
"""

import numpy as np
import sys
from contextlib import ExitStack

sys.path.insert(0, "/opt/trn_rl_repo")

import ml_dtypes
import concourse.bass as bass
import concourse.bacc as bacc
import concourse.tile as tile
from concourse import mybir
from concourse import bass_utils
from concourse import bass_isa

bf16 = ml_dtypes.bfloat16

# ---- algorithm constants (validated by numerics simulation) ----
BETA = 6.0
T_OFF = 2.5
A_SHIFT = 8.0         # downshift of the generic segment init
B, T, L = 128, 2048, 128
S = 40                # time segments per batch row
Lseg = 50             # real steps per segment
t_begin = T - S * Lseg          # 48: first device-covered real step for seg 0
DELTA = 12            # pre-roll steps for convergence
T0 = t_begin - DELTA - 1        # 35 exact host warmup steps
N = DELTA + Lseg      # 94 device steps per column
D = 6                 # renorm sample -> apply delay (steps)
SAMPLES = sorted(set(range(5, DELTA, 8)) | set(range(DELTA + 7, N - 1 - D, 20)))
NSAMP = len(SAMPLES)  # 3
BS = 16               # batch rows per core
NCORE = 8
C = BS * S            # 512 columns per core
GC = C // 2           # matmul group width
# chunk boundaries: single-step lead-in chunks to cut pipeline-fill time
CHB = [0, 1, 2, 3, 4, 6, 8]
while CHB[-1] < N:
    CHB.append(min(CHB[-1] + 8, N))
NCH = len(CHB) - 1
START_LABEL, STOP_LABEL = 126, 127

F32 = mybir.dt.float32
BF16 = mybir.dt.bfloat16

_BUILT = None  # cached (nc,)


def _build_module():
    nc = bacc.Bacc(
        "TRN2",
        target_bir_lowering=False,
        debug=False,
        enable_asserts=False,
        num_devices=NCORE,
    )
    emX_d = nc.dram_tensor("emX", [L, N, C], BF16, kind="ExternalInput")
    u0_d = nc.dram_tensor("u0", [L, C], BF16, kind="ExternalInput")
    w_d = nc.dram_tensor("wmat", [L, L], BF16, kind="ExternalInput")
    ust_d = nc.dram_tensor("ust", [L, C], BF16, kind="ExternalOutput")
    ufin_d = nc.dram_tensor("ufin", [L, C], BF16, kind="ExternalOutput")
    stash_d = nc.dram_tensor("stash", [1, NSAMP, C], F32, kind="ExternalOutput")

    with tile.TileContext(nc) as tc:
        with ExitStack() as ctx:
            singles = ctx.enter_context(tc.tile_pool(name="singles", bufs=1))
            upool = ctx.enter_context(tc.tile_pool(name="upool", bufs=4))
            stg = ctx.enter_context(tc.tile_pool(name="stg", bufs=4))
            small = ctx.enter_context(tc.tile_pool(name="small", bufs=2))
            psumP = ctx.enter_context(tc.tile_pool(name="psumP", bufs=4, space="PSUM"))

            # one-time setup
            xfull = singles.tile([L, N, C], BF16)
            stash_sb = singles.tile([1, NSAMP, C], F32)
            u_tiles = [upool.tile([L, C], BF16, tag="u", name=f"u{k}") for k in range(4)]

            emX = emX_d.ap()
            stage_tiles = {}

            def stage_chunk(k):
                if k >= NCH:
                    return
                i0, i1 = CHB[k], CHB[k + 1]
                csz = i1 - i0
                st = stg.tile([L, 8, C], BF16, tag="stage", name=f"st{k}")
                nc.sync.dma_start(out=st[:, :csz, :], in_=emX[:, i0:i1, :])
                stage_tiles[k] = (st, csz)

            def produce_chunk(k):
                if k >= NCH:
                    return
                st, csz = stage_tiles.pop(k)
                i0 = CHB[k]
                nc.scalar.activation(
                    out=xfull[:, i0:i0 + csz, :],
                    in_=st[:, :csz, :],
                    func=mybir.ActivationFunctionType.Exp,
                    scale=BETA,
                )

            # prologue: chunk0 on SP queue; u0/W via DVE queue (parallel);
            # single-step lead-in chunks produced immediately
            stage_chunk(0)
            wb = singles.tile([L, L], BF16)
            nc.gpsimd.dma_start(out=wb, in_=w_d.ap())
            nc.gpsimd.dma_start(out=u_tiles[0], in_=u0_d.ap())
            produce_chunk(0)
            stage_chunk(1)
            produce_chunk(1)
            stage_chunk(2)
            produce_chunk(2)
            stage_chunk(3)

            ridx = 0
            pending_renorm = []
            for i in range(N):
                if i > 0 and i in CHB[:-1]:
                    ch = CHB.index(i)
                    stage_chunk(ch + 3)
                    if ch + 2 in stage_tiles:
                        produce_chunk(ch + 2)
                u_cur = u_tiles[i % 4]
                u_nxt = u_tiles[(i + 1) % 4]
                for g in range(2):
                    sl = slice(g * GC, (g + 1) * GC)
                    p = psumP.tile([L, GC], F32, tag=f"P{g}")
                    nc.tensor.matmul(p, wb, u_cur[:, sl], start=True, stop=True)
                    nc.vector.tensor_tensor(
                        out=u_nxt[:, sl], in0=p, in1=xfull[:, i, sl],
                        op=mybir.AluOpType.mult)
                    if i == N - 1:
                        nc.sync.dma_start(out=ufin_d.ap()[:, sl], in_=u_nxt[:, sl])
                if i == DELTA - 1:
                    nc.sync.dma_start(out=ust_d.ap(), in_=u_nxt)
                if i in SAMPLES:
                    mbc = small.tile([L, C], F32, tag="mbc", name=f"mbc{i}")
                    nc.gpsimd.partition_all_reduce(
                        mbc, u_nxt, channels=L, reduce_op=bass_isa.ReduceOp.max)
                    nc.scalar.copy(stash_sb[:, ridx, :], mbc[0:1, :])
                    pending_renorm.append((i, mbc))
                    ridx += 1
                    if ridx == NSAMP:
                        nc.sync.dma_start(out=stash_d.ap(), in_=stash_sb)
                if pending_renorm and i == pending_renorm[0][0] + 3:
                    si, mbc = pending_renorm.pop(0)
                    rfac = small.tile([L, C], F32, tag="rfac", name=f"rfac{si}")
                    nc.vector.reciprocal(rfac, mbc)
                    nc.vector.tensor_tensor(
                        out=xfull[:, si + D, :], in0=xfull[:, si + D, :], in1=rfac,
                        op=mybir.AluOpType.mult)

            assert ridx == NSAMP, (ridx, NSAMP)

    nc.compile()
    return nc


def _mp_step(v, tr, e_t):
    # one exact max-plus step, vectorized over batch
    return (v[:, :, None] + tr[None, :, :]).max(axis=1) + e_t


def kernel(emissions: np.ndarray, transitions: np.ndarray):
    global _BUILT
    em = np.ascontiguousarray(np.asarray(emissions, dtype=np.float32))
    tr = np.ascontiguousarray(np.asarray(transitions, dtype=np.float32))
    assert em.shape == (B, T, L) and tr.shape == (L, L)

    # ---- host: exact warmup to t=T0 and c_x calibration ----
    v = np.full((B, L), -10000.0, dtype=np.float32)
    v[:, START_LABEL] = 0.0
    for t in range(1, T0 + 1):
        v = _mp_step(v, tr, em[:, t, :])
    vT0 = v.astype(np.float64)
    vmax = vT0.max(axis=1)                      # [B]

    m = em.max(axis=2).astype(np.float64)       # [B, T]
    vc = v[:4].copy()
    gains = []
    for t in range(T0 + 1, T0 + 61):
        vn = _mp_step(vc, tr, em[:4, t, :])
        gains.append((vn.max(axis=1) - vc.max(axis=1)).astype(np.float64) - m[:4, t])
        vc = vn
    c_x = float(np.mean(np.stack(gains)[10:])) - T_OFF
    mcw = m + c_x + T_OFF                       # per-step constant, f64

    # ---- build (static module, cached) ----
    if _BUILT is None:
        _BUILT = (_build_module(),)
    nc = _BUILT[0]

    # ---- per-core inputs ----
    wmat = np.exp(BETA * (tr.astype(np.float64) - T_OFF)).astype(np.float32).astype(bf16)
    emp = (em.astype(np.float64) - m[:, :, None] - c_x).astype(np.float32).astype(bf16)
    starts = np.array([t_begin + s * Lseg - DELTA for s in range(S)])

    prof = (vT0 - vmax[:, None]).mean(axis=0)
    prof = prof - prof.max() - A_SHIFT
    u_generic = np.exp(BETA * prof).astype(np.float32)      # [L]

    in_maps = []
    for core in range(NCORE):
        b0 = core * BS
        emX = np.empty((L, N, C), dtype=bf16)
        u0 = np.tile(u_generic[:, None], (1, C)).astype(np.float32)
        for s in range(S):
            t0s = starts[s]
            # columns c = b_local*S + s
            emX[:, :, s::S] = emp[b0:b0 + BS, t0s:t0s + N, :].transpose(2, 1, 0)
        for b_local in range(BS):
            u0[:, b_local * S] = np.exp(BETA * (vT0[b0 + b_local] - vmax[b0 + b_local])
                                        ).astype(np.float32)
        in_maps.append({"emX": emX, "u0": u0.astype(bf16), "wmat": wmat})

    res = bass_utils.run_bass_kernel_spmd(
        nc, in_maps, core_ids=list(range(NCORE)))

    # ---- host: stitch in f64 ----
    scores = np.zeros((B,), np.float32)
    labels = np.zeros((B,), np.int32)
    tiny = 1e-300
    n_st = sum(1 for sp in SAMPLES if sp + D <= DELTA - 1)
    for core in range(NCORE):
        out = res.results[core]
        ust = np.asarray(out["ust"], dtype=np.float64)        # [L, C]
        ufin = np.asarray(out["ufin"], dtype=np.float64)      # [L, C]
        stash = np.asarray(out["stash"], dtype=np.float64).reshape(NSAMP, C)
        lstash = np.log(np.maximum(stash, tiny))
        lf_st = lstash[:n_st].sum(axis=0)                     # [C]
        lf_fi = lstash.sum(axis=0)
        lust = np.log(np.maximum(ust, tiny)) / BETA + lf_st[None, :] / BETA
        lufi = np.log(np.maximum(ufin, tiny)) / BETA + lf_fi[None, :] / BETA
        b0 = core * BS
        for b_local in range(BS):
            b = b0 + b_local
            cs = b_local * S + np.arange(S)
            mc_st = np.array([mcw[b, starts[s]:starts[s] + DELTA].sum() for s in range(S)])
            mc_fi = np.array([mcw[b, starts[s]:starts[s] + N].sum() for s in range(S)])
            wst = lust[:, cs] + mc_st[None, :]                # [L, S]
            wfi = lufi[:, cs] + mc_fi[None, :]
            coff = np.zeros(S)
            coff[0] = -vmax[b]
            for s in range(1, S):
                a = wfi[:, s - 1]
                bb = wst[:, s]
                valid = (ufin[:, cs[s - 1]] > 1e-250) & (ust[:, cs[s]] > 1e-250)
                j = int(np.argmax(np.where(valid, a, -1e18)))
                coff[s] = coff[s - 1] + (bb[j] - a[j])
            vfin = wfi[:, S - 1] - coff[S - 1] + tr[:, STOP_LABEL].astype(np.float64)
            scores[b] = np.float32(vfin.max())
            labels[b] = np.int32(vfin.argmax())
    return scores, labels


if __name__ == "__main__":
    rng = np.random.default_rng(0)
    em = rng.standard_normal((B, T, L)).astype(np.float32)
    tr = rng.standard_normal((L, L)).astype(np.float32)
    tr[:, START_LABEL] = 0.0
    tr[STOP_LABEL, :] = 0.0
    s, l = kernel(em, tr)
    print(s[:8], l[:8])
